# revision 25
# baseline (speedup 1.0000x reference)
"""Multi-head causal self-attention on 8 TRN2 NeuronCores.

Problem: B=2, T=4096, D=512, H=8 heads (hd=64), fp32 in/out.

Sharding: core c in 0..7 handles batch b = c//4 and head pair g = c%4
(heads 2g, 2g+1 -> D-slice [128g, 128g+128)). Each core computes
    partial_out = concat_h( softmax(causal(Q_h K_h^T / 8)) V_h ) @ W_O[slice]
for its two heads; the host sums the 4 partials per batch and adds b_O.

On-core dataflow (all matmul operands bf16, f32 PSUM accumulation):
  - X^T streams in as 32 [128,512] tiles, issued in slice order across two
    DMA queues (SP + Pool) so slice-0 compute starts ~3us in.
  - Q^T,K^T [128(d-pair),4096] = W_chunk^T @ X^T, bias added during the
    PSUM->SBUF bf16 copy (per-partition scalar add on DVE). QKV for slice
    s+1 is emitted while ScalarE still exps slice s's last group, so the
    PE never idles at slice boundaries.
  - V per key block in natural layout [128, VA(64)|1|VB(64)|1]; the ones
    columns make the attention row-sum L fall out of the PV matmul for
    free; b_V is added via a pre-replicated SBUF tile fused into the DVE
    evacuation (no PE bias matmuls).
  - Scores are computed transposed, S^T[k-block, q] (contraction over the
    64-dim head axis; the two heads occupy disjoint PE row groups),
    causally streamed: for key block kb only q >= 128*kb is computed.
    exp() runs on ScalarE straight out of PSUM with the 1/8 scale folded
    in; the diagonal 128x128 subtile is masked by accumulating -1e9
    upper-triangle via an identity matmul before the exp.
  - Z^T_aug[65, q] accumulates P^T-block x V_aug over key blocks in PSUM;
    row 64 is L. The S->exp->PV chain is software-pipelined one group
    ahead; score-group PSUM tiles are double-buffered and the Z
    accumulators double-buffered across slices.
  - Normalisation (off the critical path): r = 1/L via the single-op
    approximate DVE reciprocal, broadcast across partitions with a rank-1
    PE matmul (ones[1,64]^T @ r[1,512] -> PSUM), then one DVE multiply
    evacuates Z^T * r to bf16 SBUF. No DRAM bounce.
  - O-projection: lhsT = stacked [Z_A; Z_B] [128, t-tile] (head B shifted
    to partitions 64..127 via SBUF->SBUF DMA), rhs = W_O pair [128,512].
    Each slice's O-projection is emitted under the NEXT slice's attention
    groups so its normalisation chain never stalls the PE stream.
"""

import numpy as np

import concourse.bass as bass
import concourse.mybir as mybir
from concourse.tile import TileContext
from concourse.bass_utils import run_bass_kernel_spmd

try:
    import ml_dtypes

    _BF16 = ml_dtypes.bfloat16
except ImportError:  # pragma: no cover
    _BF16 = None

F32 = mybir.dt.float32
BF16 = mybir.dt.bfloat16

B, T, D, H = 2, 4096, 512, 8
HD = D // H  # 64
SW = 512  # q-slice width
NS = T // SW  # 8 q-slices
NKC = D // 128  # 4 contraction chunks for the projections
NTT = T // 128  # 32 t-tiles / key blocks
GK = 2  # key blocks grouped per exp() call (2 PSUM banks)
NEG = -1.0e9


def _split_waits(nc, max_waits=1):
    """The staged walrus rejects >1 semaphore wait per instruction; hoist
    extras onto same-engine NoOps inserted right before the instruction."""
    counter = 0
    for f in nc.m.functions:
        for blk in f.blocks:
            insts = blk.instructions
            out, changed = [], False
            for ins in insts:
                si = getattr(ins, "sync_info", None)
                waits = list(si.on_wait) if si is not None and si.on_wait else []
                if len(waits) > max_waits:
                    changed = True
                    for w in waits[:-max_waits]:
                        counter += 1
                        nop = mybir.InstNoOp(
                            name=f"I-wsplit-{counter}",
                            engine=ins.engine,
                            ins=[],
                            outs=[],
                        )
                        nop.sync_info = mybir.SyncInfo(on_wait=[w], on_update=[])
                        out.append(nop)
                    ins.sync_info = mybir.SyncInfo(
                        on_wait=waits[-max_waits:], on_update=list(si.on_update)
                    )
                out.append(ins)
            if changed:
                blk.instructions = out
    return counter


def build_nc():
    nc = bass.Bass("TRN2")

    xt = nc.dram_tensor("xt", [D, T], BF16, kind="ExternalInput")
    # w{q,k,v} host-packed [128, NKC*128]: chunk c of the [D,128] column
    # slice lives at cols [c*128,(c+1)*128)
    wq = nc.dram_tensor("wq", [128, D], BF16, kind="ExternalInput")
    wk = nc.dram_tensor("wk", [128, D], BF16, kind="ExternalInput")
    wv = nc.dram_tensor("wv", [128, D], BF16, kind="ExternalInput")
    wo = nc.dram_tensor("wo", [128, D], BF16, kind="ExternalInput")
    bq = nc.dram_tensor("bq", [128, 1], F32, kind="ExternalInput")
    bk = nc.dram_tensor("bk", [128, 1], F32, kind="ExternalInput")
    bv = nc.dram_tensor("bv", [1, 128], BF16, kind="ExternalInput")
    out = nc.dram_tensor("out", [T, D], F32, kind="ExternalOutput")

    ident_np = np.eye(128, dtype=np.float32)
    # maskneg[k, q'] = 0 where q' >= k else NEG  (S^T diagonal subtile mask)
    mask_np = np.where(
        np.arange(128)[None, :] >= np.arange(128)[:, None], 0.0, NEG
    ).astype(np.float32)
    ident_dram = nc.inline_tensor(ident_np.astype(_BF16), name="identc")
    mask_dram = nc.inline_tensor(mask_np.astype(_BF16), name="maskc")

    with TileContext(nc) as tc:
        with (
            tc.tile_pool(name="singles", bufs=1) as singles,
            tc.tile_pool(name="ps", bufs=3, space="PSUM") as ps,
            tc.tile_pool(name="zps", bufs=1, space="PSUM") as zps,
            tc.tile_pool(name="pt", bufs=6) as ptp,
            tc.tile_pool(name="sl", bufs=3) as slp,
            tc.tile_pool(name="outp", bufs=6) as outp,
            tc.tile_pool(name="drp", bufs=2, space="DRAM") as drp,
        ):
            # ---- static SBUF tiles ----
            xt_sb = [
                [
                    singles.tile(
                        [128, SW], BF16, tag=f"xt{c}_{s}", name=f"xt_sb{c}_{s}"
                    )
                    for s in range(NS)
                ]
                for c in range(NKC)
            ]
            wq_sb = singles.tile([128, D], BF16, tag="wq")
            wk_sb = singles.tile([128, D], BF16, tag="wk")
            wv_sb = singles.tile([128, D], BF16, tag="wv")
            wo_sb = singles.tile([128, D], BF16, tag="wo")
            bq_sb = singles.tile([128, 1], F32, tag="bq")
            bk_sb = singles.tile([128, 1], F32, tag="bk")
            bv_sb = singles.tile([128, 128], BF16, tag="bv")
            ident_sb = singles.tile([128, 128], BF16, tag="ident")
            mask_sb = singles.tile([128, 128], BF16, tag="mask")
            wob0_sb = singles.tile([HD, D], BF16, tag="wob0")

            # DMA issue order = data-need order. Slice-0-critical first on SP.
            nc.sync.dma_start(out=wq_sb[:, :], in_=wq[:, :])
            nc.sync.dma_start(out=wk_sb[:, :], in_=wk[:, :])
            for c in range(NKC):
                nc.sync.dma_start(
                    out=xt_sb[c][0][:, :], in_=xt[c * 128 : (c + 1) * 128, 0:SW]
                )
            nc.sync.dma_start(out=bq_sb[:, :], in_=bq[:, :])
            nc.sync.dma_start(out=bk_sb[:, :], in_=bk[:, :])
            nc.sync.dma_start(out=wv_sb[:, :], in_=wv[:, :])
            # bv replicated across partitions (step-0 partition AP, DRAM src)
            bvap = bv[:, :]
            nc.sync.dma_start(
                out=bv_sb[:, :],
                in_=bass.AP(
                    tensor=bvap.tensor,
                    offset=bvap.offset,
                    ap=[[0, 128]] + list(bvap.ap[1:]),
                ),
            )
            nc.sync.dma_start(out=ident_sb[:, :], in_=ident_dram[:, :])
            nc.sync.dma_start(out=mask_sb[:, :], in_=mask_dram[:, :])
            for s in range(1, 4):
                for c in range(NKC):
                    nc.sync.dma_start(
                        out=xt_sb[c][s][:, :],
                        in_=xt[c * 128 : (c + 1) * 128, s * SW : (s + 1) * SW],
                    )
            nc.sync.dma_start(out=wo_sb[:, :], in_=wo[:, :])
            # head-B rows of W_O re-homed at partitions 0..63 for the
            # last slice's per-head O-projection
            nc.sync.dma_start(out=wob0_sb[:, :], in_=wo[HD:128, :])
            for s in range(4, 6):
                for c in range(NKC):
                    nc.sync.dma_start(
                        out=xt_sb[c][s][:, :],
                        in_=xt[c * 128 : (c + 1) * 128, s * SW : (s + 1) * SW],
                    )
            for s in range(6, NS):
                for c in range(NKC):
                    nc.gpsimd.dma_start(
                        out=xt_sb[c][s][:, :],
                        in_=xt[c * 128 : (c + 1) * 128, s * SW : (s + 1) * SW],
                    )

            qt_sb = [
                singles.tile([128, SW], BF16, tag=f"qt{s}", name=f"qt_sb{s}")
                for s in range(NS)
            ]
            kt_sb = [
                singles.tile([128, SW], BF16, tag=f"kt{s}", name=f"kt_sb{s}")
                for s in range(NS)
            ]
            # V_aug per key block: [128(t), VA(64) | 1 | VB(64) | 1]
            v_sb = [
                singles.tile([128, 2 * HD + 2], BF16, tag=f"v{t}", name=f"v_sb{t}")
                for t in range(NTT)
            ]

            # ---- emission helpers ----
            def emit_qk(s):
                ps_q = ps.tile([128, SW], F32, tag="sg", name="ps_q")
                for c in range(NKC):
                    nc.tensor.matmul(
                        ps_q[:, :],
                        lhsT=wq_sb[:, c * 128 : (c + 1) * 128],
                        rhs=xt_sb[c][s][:, :],
                        start=(c == 0),
                        stop=(c == NKC - 1),
                        skip_group_check=True,
                    )
                nc.vector.tensor_scalar_add(qt_sb[s][:, :], ps_q[:, :], bq_sb[:, :])
                ps_k = ps.tile([128, SW], F32, tag="sg", name="ps_k")
                for c in range(NKC):
                    nc.tensor.matmul(
                        ps_k[:, :],
                        lhsT=wk_sb[:, c * 128 : (c + 1) * 128],
                        rhs=xt_sb[c][s][:, :],
                        start=(c == 0),
                        stop=(c == NKC - 1),
                        skip_group_check=True,
                    )
                nc.vector.tensor_scalar_add(kt_sb[s][:, :], ps_k[:, :], bk_sb[:, :])

            def emit_v(s):
                ps_v = ps.tile([128, 2 * SW], F32, tag="sg", name="ps_v")
                for t in range(4 * s, 4 * s + 4):
                    tloc = slice((t % 4) * 128, (t % 4 + 1) * 128)
                    vcol = slice((t % 4) * 128, (t % 4) * 128 + 128)
                    for c in range(NKC):
                        nc.tensor.matmul(
                            ps_v[:, vcol],
                            lhsT=xt_sb[c][s][:, tloc],
                            rhs=wv_sb[:, c * 128 : (c + 1) * 128],
                            start=(c == 0),
                            stop=(c == NKC - 1),
                            skip_group_check=True,
                        )
                for t in range(4 * s, 4 * s + 4):
                    c0 = (t % 4) * 128
                    # fused +b_V during evacuation; ones cols at 64 and 129
                    nc.vector.scalar_tensor_tensor(
                        v_sb[t][:, 0:HD],
                        ps_v[:, c0 : c0 + HD],
                        1.0,
                        bv_sb[:, 0:HD],
                        op0=mybir.AluOpType.mult,
                        op1=mybir.AluOpType.add,
                    )
                    nc.vector.scalar_tensor_tensor(
                        v_sb[t][:, HD + 1 : 2 * HD + 1],
                        ps_v[:, c0 + HD : c0 + 128],
                        1.0,
                        bv_sb[:, HD:128],
                        op0=mybir.AluOpType.mult,
                        op1=mybir.AluOpType.add,
                    )
                    nc.vector.memset(v_sb[t][:, HD : HD + 1], 1.0)
                    nc.vector.memset(v_sb[t][:, 2 * HD + 1 : 2 * HD + 2], 1.0)

            vcols = (slice(0, HD + 1), slice(HD + 1, 2 * HD + 2))
            hrows = (slice(0, HD), slice(HD, 128))

            def emit_oproj(znpair_t, qs_t):
                for j in range(4):
                    ps_o = ps.tile([128, 2 * SW], F32, tag="sg", name="ps_o")
                    nc.tensor.matmul(
                        ps_o[:, 0:D],
                        lhsT=znpair_t[:, j * 128 : (j + 1) * 128],
                        rhs=wo_sb[:, :],
                        start=True,
                        stop=True,
                        skip_group_check=True,
                    )
                    o_sb = outp.tile([128, D], F32, tag="ot", name="o_sb")
                    nc.vector.tensor_copy(o_sb[:, :], ps_o[:, 0:D])
                    r0 = qs_t + j * 128
                    nc.sync.dma_start(out=out[r0 : r0 + 128, :], in_=o_sb[:, :])

            # ---- main loop (scores->exp->PV pipelined ACROSS slices: the
            # previous slice's last PV + normalisation + O-projections are
            # emitted under the next slice's first score groups) ----
            emit_qk(0)
            emit_v(0)
            pending = []
            av_queue = []  # (pt, grp, zaug, nkb, qs)

            def emit_av(av):
                pt_t, grp_t, zaug_t, nkb_t, qs_t = av
                for h in range(2):
                    for kb, off, n, qlo in grp_t:
                        nc.tensor.matmul(
                            zaug_t[h][0 : HD + 1, qlo - qs_t : SW],
                            lhsT=v_sb[kb][:, vcols[h]],
                            rhs=pt_t[h][:, off : off + n],
                            start=(kb == 0),
                            stop=(kb == nkb_t - 1),
                            skip_group_check=True,
                        )

            def emit_norm(zaug, qs):
                # normalisation: evacuate Z_aug (frees the PSUM bank),
                # 1/L on a [128,8] partition-spread via one DRAM round-trip
                # shared by both heads, broadcast back with a step-0
                # partition DMA read (legal from DRAM), one multiply
                zsb = [None, None]
                for h in range(2):
                    zsb[h] = slp.tile([HD + 1, SW], F32, tag=f"zsb{h}", name="zsb")
                    nc.vector.tensor_copy(zsb[h][:, :], zaug[h][:, :])
                rd = drp.tile([1, 2 * SW], F32, tag="rd", name="rd")
                for h in range(2):
                    nc.sync.dma_start(
                        out=rd[:, h * SW : (h + 1) * SW], in_=zsb[h][HD : HD + 1, :]
                    )
                lsp = slp.tile([128, 2 * SW // 128], F32, tag="lsp", name="lsp")
                nc.sync.dma_start(
                    out=lsp[:, :], in_=rd[0, :].rearrange("(p f) -> p f", p=128)
                )
                rsp = slp.tile([128, 2 * SW // 128], F32, tag="rsp", name="rsp")
                nc.vector.reciprocal(rsp[:, :], lsp[:, :])
                rd2 = drp.tile([1, 2 * SW], F32, tag="rd2", name="rd2")
                nc.sync.dma_start(
                    out=rd2[0, :].rearrange("(p f) -> p f", p=128), in_=rsp[:, :]
                )
                bc_sb = slp.tile([HD, 2 * SW], F32, tag="bcs", name="bc_sb")
                rap = rd2[:, :]
                nc.sync.dma_start(
                    out=bc_sb[:, :],
                    in_=bass.AP(
                        tensor=rap.tensor,
                        offset=rap.offset,
                        ap=[[0, HD]] + list(rap.ap[1:]),
                    ),
                )
                znpair = slp.tile([128, SW], BF16, tag="zn")
                znb = slp.tile([HD, SW], BF16, tag="znb")
                for h in range(2):
                    dst = znpair[0:HD, :] if h == 0 else znb[:, :]
                    nc.vector.tensor_mul(
                        dst, zsb[h][0:HD, :], bc_sb[:, h * SW : (h + 1) * SW]
                    )
                # move head B rows into partitions 64..127
                nc.gpsimd.dma_start(out=znpair[HD:128, :], in_=znb[:, :])
                pending.append((znpair, qs))

            prev = None  # previous slice's (zaug, qs) awaiting normalisation
            for s in range(NS):
                qs = s * SW
                nkb = 4 * (s + 1)
                zaug = [
                    zps.tile([HD + 1, SW], F32, tag="za", name="zauga"),
                    zps.tile([HD + 1, SW], F32, tag="zb", name="zaugb"),
                ]
                # pack key blocks tightly into groups; a matmul output may
                # not cross a PSUM bank boundary, so bump to the next bank
                # when a block would straddle one
                groups, cur, cur_cols = [], [], 0
                for kb in range(nkb):
                    qlo = max(qs, kb * 128)
                    n = qs + SW - qlo
                    off = cur_cols
                    if off % SW + n > SW:
                        off = ((off + SW - 1) // SW) * SW
                    if off + n > GK * SW:
                        groups.append(cur)
                        cur, off = [], 0
                    cur.append((kb, off, n, qlo))
                    cur_cols = off + n
                if cur:
                    groups.append(cur)

                for gi, grp in enumerate(groups):
                    used = grp[-1][1] + grp[-1][2]
                    sg = [None, None]
                    pt = [None, None]
                    for h in range(2):
                        sg[h] = ps.tile([128, GK * SW], F32, tag="sg", name="sg")
                        pt[h] = ptp.tile([128, GK * SW], BF16, tag="pt", name="pt")
                    # scores (both heads interleaved -> disjoint PE row groups)
                    for kb, off, n, qlo in grp:
                        diag = kb * 128 >= qs
                        for h in range(2):
                            nc.tensor.matmul(
                                sg[h][:, off : off + n],
                                lhsT=kt_sb[kb // 4][
                                    hrows[h], (kb % 4) * 128 : (kb % 4 + 1) * 128
                                ],
                                rhs=qt_sb[s][hrows[h], qlo - qs : qlo - qs + n],
                                start=True,
                                stop=not diag,
                                skip_group_check=True,
                                tile_position=(h * HD, 0),
                            )
                        if diag:
                            for h in range(2):
                                nc.tensor.matmul(
                                    sg[h][:, off : off + 128],
                                    lhsT=ident_sb[:, :],
                                    rhs=mask_sb[:, :],
                                    start=False,
                                    stop=True,
                                    skip_group_check=True,
                                )
                    for h in range(2):
                        nc.scalar.activation(
                            out=pt[h][:, 0:used],
                            in_=sg[h][:, 0:used],
                            func=mybir.ActivationFunctionType.Exp,
                            scale=0.125,
                        )
                    av_queue.append((pt, grp, zaug, nkb, qs))
                    if gi == len(groups) - 1 and s + 1 < NS:
                        # keep the PE fed while ScalarE exps the last group
                        emit_qk(s + 1)
                    while len(av_queue) > 1:
                        emit_av(av_queue.pop(0))
                    if gi == 0 and prev is not None:
                        # previous slice's last PV just drained above: its
                        # normalisation + an older O-projection + this
                        # slice's V all run under this slice's attention
                        pz, pq = prev
                        prev = None
                        emit_norm(pz, pq)
                        while len(pending) > 1:
                            emit_oproj(*pending.pop(0))
                        emit_v(s)
                prev = (zaug, qs)

            # ---- tail: last slice's PV, remaining O-projections, then the
            # per-head O-projection scaled at PSUM eviction by the
            # partition-spread reciprocal (no bounce, no head-B shift) ----
            while pending:
                emit_oproj(*pending.pop(0))
            while av_queue:
                emit_av(av_queue.pop(0))
            zaug, qs = prev
            znu = [None, None]
            lr2 = slp.tile([1, 2 * SW], F32, tag="lr2", name="lr2")
            for h in range(2):
                nc.vector.tensor_copy(
                    lr2[:, h * SW : (h + 1) * SW], zaug[h][HD : HD + 1, :]
                )
            rdl = drp.tile([1, 2 * SW], F32, tag="rdl", name="rdl")
            nc.gpsimd.dma_start(out=rdl[:, :], in_=lr2[:, :])
            lspl = slp.tile([128, 2 * SW // 128], F32, tag="lsp", name="lspl")
            # column-major spread: lspl[p, f] = L[f*128 + p], so
            # rspl[:, j] is exactly 1/L for q-subtile j, per-partition
            nc.gpsimd.dma_start(
                out=lspl[:, :], in_=rdl[0, :].rearrange("(f p) -> p f", p=128)
            )
            for h in range(2):
                znu[h] = slp.tile([HD, SW], BF16, tag=f"znu{h}", name="znu")
                nc.vector.tensor_copy(znu[h][:, :], zaug[h][0:HD, :])
            rspl = slp.tile([128, 2 * SW // 128], F32, tag="rsp", name="rspl")
            nc.vector.reciprocal(rspl[:, :], lspl[:, :])
            for j in range(4):
                ps_a = ps.tile([128, 2 * SW], F32, tag="sg", name="ps_oa")
                nc.tensor.matmul(
                    ps_a[:, 0:D],
                    lhsT=znu[0][:, j * 128 : (j + 1) * 128],
                    rhs=wo_sb[0:HD, :],
                    start=True,
                    stop=True,
                    skip_group_check=True,
                )
                ps_b = ps.tile([128, 2 * SW], F32, tag="sg", name="ps_ob")
                nc.tensor.matmul(
                    ps_b[:, 0:D],
                    lhsT=znu[1][:, j * 128 : (j + 1) * 128],
                    rhs=wob0_sb[:, :],
                    start=True,
                    stop=True,
                    skip_group_check=True,
                )
                o_sb = outp.tile([128, D], F32, tag="ot", name="o_sb")
                nc.vector.tensor_scalar_mul(
                    o_sb[:, :], ps_a[:, 0:D], rspl[:, j : j + 1]
                )
                nc.vector.scalar_tensor_tensor(
                    o_sb[:, :],
                    ps_b[:, 0:D],
                    rspl[:, 4 + j : 5 + j],
                    o_sb[:, :],
                    op0=mybir.AluOpType.mult,
                    op1=mybir.AluOpType.add,
                )
                r0 = qs + j * 128
                nc.sync.dma_start(out=out[r0 : r0 + 128, :], in_=o_sb[:, :])

    _split_waits(nc)
    return nc


_NC_CACHE = {}


def _get_nc():
    if "nc" not in _NC_CACHE:
        _NC_CACHE["nc"] = build_nc()
    return _NC_CACHE["nc"]


def make_in_maps(combined_embed, W_K, b_K, W_Q, b_Q, W_V, b_V, W_O, b_O):
    f32 = np.float32

    def packw(W, sl):
        # [D,128] column slice -> [128, NKC*128] with chunk c at cols c*128+
        Wc = np.asarray(W, f32)[:, sl]
        return np.ascontiguousarray(
            np.concatenate([Wc[c * 128 : (c + 1) * 128, :] for c in range(NKC)], 1)
        ).astype(_BF16)

    in_maps = []
    for c in range(8):
        b = c // 4
        g = c % 4
        sl = slice(g * 128, (g + 1) * 128)
        xt = np.ascontiguousarray(np.asarray(combined_embed[b], f32).T)
        in_maps.append(
            {
                "xt": xt.astype(_BF16),
                "wq": packw(W_Q, sl),
                "wk": packw(W_K, sl),
                "wv": packw(W_V, sl),
                "wo": np.ascontiguousarray(np.asarray(W_O, f32)[sl, :]).astype(_BF16),
                "bq": np.asarray(b_Q, f32)[sl].reshape(128, 1).copy(),
                "bk": np.asarray(b_K, f32)[sl].reshape(128, 1).copy(),
                "bv": np.asarray(b_V, f32)[sl].reshape(1, 128).astype(_BF16),
            }
        )
    return in_maps


def run_cores(in_maps, **kwargs):
    nc = _get_nc()
    return run_bass_kernel_spmd(nc, in_maps, core_ids=list(range(8)), **kwargs)


def kernel(
    combined_embed, W_K, b_K, W_Q, b_Q, W_V, b_V, W_O, b_O
):  # full inputs -> full output
    in_maps = make_in_maps(
        combined_embed, W_K, b_K, W_Q, b_Q, W_V, b_V, W_O, b_O
    )
    res = run_cores(in_maps)
    out = np.zeros((B, T, D), np.float32)
    for c in range(8):
        out[c // 4] += res.results[c]["out"]
    out += np.asarray(b_O, np.float32)[None, None, :]
    return out
